# revision 33
# baseline (speedup 1.0000x reference)
"""BiDAF attention kernel for Trainium2 (8 NeuronCores, data-parallel over batch).

Problem (per full input): B=16, L=M=1024, H=128
  s  = text@tw + (mod@mw).T + (text*tmw)@mod.T + bias          (B, L, M)
  p1 = softmax_M(mmask*s + (1-mmask)*NEG)
  p2 = softmax_L(tmask*s + (1-tmask)*NEG)
  a  = p1 @ mod
  b  = p1 @ p2.T @ text        (computed as p1 @ (p2.T @ text))
  out = [text, a, text*a, text*b]                               (B, L, 4H)

Key facts used:
  * softmax_M is invariant to per-row (per-l) shifts: s0, bias drop from p1;
    softmax_L is invariant to per-column shifts: s1, bias drop from p2.
    The scalar `bias` input therefore cannot affect the output at all.
  * exp(s2 + b) factors: the per-l bias b2 = s0 + (tmask-1)*30000 enters p2
    only through q2 = E2.T @ [textg|1], where E2's partition dim (lg) is the
    contraction dim -- so exp(b2) folds into the rhs rows (host pre-scales
    [textg|1] by exp(b2); masked rows become exactly 0).  Likewise the per-m
    bias b1 = s1 + (mmask-1)*30000 enters the final matmul through its lhsT
    partition dim (mg), so exp(b1) folds into the [mod|wq|1] rows (host
    pre-scales; masked m rows become 0; the device-computed wq columns get
    the factor via a fused dual-scalar multiply).  All device exps are then
    bias-free pure exp(s2), which lets chunks share one ACTIVATE (the ~480ns
    fixed cost per ACTIVATE dominates otherwise).
  * a ones-column on the rhs of the q2/final matmuls yields the softmax
    denominators for free; D1 normalization + text*a / text*b are O(L*H)
    epilogue done on the host along with all layout prep (gather/compaction,
    transposes, bf16 casts, w_tm pre-scale).  The device runs the O(L*M(*H))
    work: 4 matmul families and the exps.
  * sparsity: masked m/l rows are compacted away host-side; the device
    computes only ceil(Mu/128) x ceil(Lu/128) chunks.

Device program per batch (matmul operands bf16; exp psum bf16, accum f32):
  E2[lg, mg]  = exp(XgT_c.T @ modTg)           chunk-pair-merged ACTIVATEs
  E1T[mg, l]  = exp(modTg_k.T @ txtT)          chunk-pair-merged ACTIVATEs
  q2[mg, :]   = sum_c E2_c.T @ [textg*eb2|eb2] 25 matmuls N=129
  wq          = q2[:, :H] * (1/D2) * eb1       DVE fused dual-scalar
  [ar|br|D1]  = sum_k E1_k @ [mod*eb1|wq*eb1|eb1]  k-outer over 4-j groups
Raw [ar|br|D1] (bf16) is DMA'd out; host divides by D1 and assembles
[text, a, t*a, t*b].  Each core processes 2 batches; no cross-core comm.
"""

import numpy as np
import ml_dtypes

BF16 = ml_dtypes.bfloat16
B, L, M, H = 16, 1024, 1024, 128
NCORES = 8
BPC = B // NCORES  # batches per core
P = 128
LT = L // P
NEGB = 30000.0
NO = 2 * H + 1  # raw output row: [a_raw | b_raw | D1]

_CACHE = {}


def _build(MU, LU):
    """Per-core Bass program for MU gathered m-chunks / LU gathered l-chunks
    (SPMD: same NEFF on all 8 cores)."""
    from contextlib import ExitStack

    import concourse.bass as bass
    import concourse.mybir as mybir
    import concourse.tile as tile
    from concourse import bacc
    from concourse.bass import ts

    f32 = mybir.dt.float32
    bf16 = mybir.dt.bfloat16
    Exp = mybir.ActivationFunctionType.Exp
    Alu = mybir.AluOpType

    MG, LG = MU * P, LU * P
    NC1 = H + 1      # [textg | 1] * exp(b2)
    NC2 = 2 * H + 1  # [mod | wq | 1] * exp(b1)

    nc = bacc.Bacc(name="bidaf8v3")
    txtT_d = nc.dram_tensor("txtT", (BPC, H, L), bf16, kind="ExternalInput").ap()
    crit_d = nc.dram_tensor("crit", (BPC, H, LG + MG), bf16,
                            kind="ExternalInput").ap()
    gath_d = nc.dram_tensor("gath", (BPC, P, LU * NC1 + MU * NC2), bf16,
                            kind="ExternalInput").ap()
    eb1_d = nc.dram_tensor("eb1", (BPC, P, MU), f32, kind="ExternalInput").ap()
    out_d = nc.dram_tensor("out_ab", (BPC, P, LT, NO), bf16,
                           kind="ExternalOutput").ap()

    with tile.TileContext(nc) as tc, ExitStack() as ctx:
        inp = ctx.enter_context(tc.tile_pool(name="inp", bufs=2))
        ebuf = ctx.enter_context(tc.tile_pool(name="ebuf", bufs=2))
        small = ctx.enter_context(tc.tile_pool(name="small", bufs=2))
        outp = ctx.enter_context(tc.tile_pool(name="outp", bufs=2))
        # One PSUM shape everywhere: (128, 1024) f32 = 2 banks; two tags x
        # 2 bufs = all 8 banks.  Each chunk/qp/pa-group gets its OWN tile so
        # the tile tracker sees precise deps (a shared multi-slot tile
        # serializes consumers against later producers).  Every matmul slice
        # below lands within a single bank.
        ps = ctx.enter_context(tc.tile_pool(name="ps", bufs=2, space="PSUM"))

        # Load the exp spline tables while the input DMAs run.
        dumm = small.tile([P, 1], f32, tag="dummy")
        nc.vector.memset(dumm, 0.0)
        dum2 = small.tile([P, 1], f32, tag="dummy2")
        nc.scalar.activation(dum2, dumm, Exp)

        st = [{} for _ in range(BPC)]
        # Few, large input DMAs so completion-sem lanes stay clean (early
        # consumers must never gate on an unrelated late transfer sharing a
        # lane).  Tiny eb1s go first; then the critical [modTg|xgT] blob,
        # split so E2 chunk 0's operands (modTg + xgT chunk 0) land first.
        # Warm the PE clock gate (HAM) during the input-DMA wait so the
        # first real fills run at 2.4 GHz.
        wsrc = small.tile([P, 64], bf16, tag="wsrc")
        nc.vector.memset(wsrc, 0.0)
        warm = ps.tile([P, 512], f32, tag="qp")
        for _ in range(30):
            nc.tensor.matmul(warm[:64, :64], wsrc, wsrc, start=True, stop=True)

        # Per-batch inputs ride one HWDGE ring each, in consumption order
        # (ring FIFO serializes them, so the critical blob is never
        # bandwidth-starved by later transfers).  Tiny eb1s go via SWDGE.
        NG = LU * NC1 + MU * NC2
        for b in range(BPC):
            d = st[b]
            eng = nc.sync if b == 0 else nc.scalar
            d["crit"] = inp.tile([P, LG + MG], bf16, tag="crit", name="crit")
            d["modTg"] = d["crit"][:, :MG]
            d["xgT"] = d["crit"][:, MG:]
            eng.dma_start(d["crit"], crit_d[b])
            d["txtT"] = inp.tile([P, L], bf16, tag="txtT", name="txtT")
            eng.dma_start(d["txtT"], txtT_d[b])
            gath = inp.tile([P, NG], bf16, tag="gath", name="gath")
            d["txtg1"] = gath[:, : LU * NC1].rearrange("p (c q) -> p c q",
                                                       q=NC1)
            d["modwq"] = gath[:, LU * NC1 :].rearrange("p (c q) -> p c q",
                                                       q=NC2)
            eng.dma_start(gath, gath_d[b])
            d["eb1"] = small.tile([P, MU], f32, tag="eb1", name="eb1")
            nc.gpsimd.dma_start(d["eb1"], eb1_d[b])

        def e2_chunk(b, c):
            d = st[b]
            sp = ps.tile([P, 1024], f32, tag="g")
            n0 = min(512, MG)
            nc.tensor.matmul(sp[:, :n0], d["xgT"][:, ts(c, P)],
                             d["modTg"][:, :n0], start=True, stop=True)
            if MG > 512:
                nc.tensor.matmul(sp[:, 512:MG], d["xgT"][:, ts(c, P)],
                                 d["modTg"][:, 512:MG], start=True, stop=True)
            nc.scalar.activation(d["E2"][:, c, :], sp[:, :MG], Exp)

        def e1t_chunk(b, k, split=False):
            # split=True: exp the two l-halves as separate tiles/ACTs so
            # final-phase consumers of the first half unblock earlier.
            d = st[b]
            if split:
                for half in range(2):
                    sp = ps.tile([P, 1024], f32, tag="g")
                    nc.tensor.matmul(sp[:, :512], d["modTg"][:, ts(k, P)],
                                     d["txtT"][:, ts(half, 512)],
                                     start=True, stop=True)
                    nc.scalar.activation(
                        d["E1T"][:, k, ts(half, 512)], sp[:, :512], Exp)
                return
            sp = ps.tile([P, 1024], f32, tag="g")
            for half in range(2):
                nc.tensor.matmul(sp[:, ts(half, 512)], d["modTg"][:, ts(k, P)],
                                 d["txtT"][:, ts(half, 512)],
                                 start=True, stop=True)
            nc.scalar.activation(d["E1T"][:, k, :], sp, Exp)

        def exp_phase(b):
            # Interleave E2/E1T chunks: the 2-deep 'g' slot rotation then
            # forces fills in exactly the ACT consumption order, keeping the
            # ACT stream (the spine of this kernel) gap-free.
            d = st[b]
            d["E2"] = ebuf.tile([P, LU, MG], bf16, tag="E2", name="E2")
            d["E1T"] = ebuf.tile([P, MU, L], bf16, tag="E1T", name="E1T")
            for c in range(LU):
                e2_chunk(b, c)
            for k in range(MU):
                e1t_chunk(b, k, split=(b == BPC - 1 and k == MU - 1))

        def q2_k(b, k):
            # q2 = E2.T @ [textg|1]*eb2 ; wq = q2 * (1/D2) * eb1
            d = st[b]
            qp = ps.tile([P, 512], f32, tag="qp")
            for c in range(LU):
                nc.tensor.matmul(qp[:, :NC1], d["E2"][:, c, ts(k, P)],
                                 d["txtg1"][:, c, :],
                                 start=(c == 0), stop=(c == LU - 1))
            rec = small.tile([P, 1], f32, tag="rec2")
            nc.vector.reciprocal(rec, qp[:, H : H + 1])
            nc.vector.tensor_scalar(d["modwq"][:, k, H : 2 * H],
                                    qp[:, :H], rec,
                                    d["eb1"][:, k : k + 1],
                                    op0=Alu.mult, op1=Alu.mult)

        Copy = mybir.ActivationFunctionType.Copy

        def fin_group(b, j, act_copy=False):
            # [ar|br|D1] = E1 @ [mod|wq|1]*eb1 for one j
            d = st[b]
            pa = ps.tile([P, 512], f32, tag="pa")
            for k in range(MU):
                nc.tensor.matmul(pa[:, :NC2], d["E1T"][:, k, ts(j, P)],
                                 d["modwq"][:, k, :],
                                 start=(k == 0), stop=(k == MU - 1))
            if act_copy:
                nc.scalar.activation(d["absb"][:, j, :], pa[:, :NO], Copy)
            else:
                nc.vector.tensor_copy(d["absb"][:, j, :], pa[:, :NO])
            nc.sync.dma_start(out_d[b][:, j, :], d["absb"][:, j, :])

        for b in range(BPC):
            st[b]["absb"] = outp.tile([P, LT, NO], bf16, tag="absb",
                                      name="absb")

        # Chain-pipelined emission: batch 0's q2/final fills PE idle while
        # batch 1's exps run on ACT, and vice versa.
        exp_phase(0)
        for k in range(MU):
            q2_k(0, k)
        exp_phase(1)
        for j in range(LT):
            fin_group(0, j)
        for k in range(MU):
            q2_k(1, k)
        for j in range(LT):
            fin_group(1, j)
    nc.compile()
    return nc


def get_nc(MU, LU):
    key = (MU, LU)
    if key not in _CACHE:
        _CACHE[key] = _build(MU, LU)
    return _CACHE[key]


def make_in_maps(text, modality, text_mask, modality_mask,
                 text_weight, modality_weight, text_modality_weight):
    text = np.ascontiguousarray(np.asarray(text, dtype=np.float32))
    modality = np.ascontiguousarray(np.asarray(modality, dtype=np.float32))
    text_mask = np.asarray(text_mask).astype(np.int32)
    modality_mask = np.asarray(modality_mask).astype(np.int32)
    wt = np.asarray(text_weight, dtype=np.float32).reshape(H)
    wm = np.asarray(modality_weight, dtype=np.float32).reshape(H)
    wtm = np.asarray(text_modality_weight, dtype=np.float32).reshape(H)

    LU = max(1, int(-(-int(text_mask.sum(axis=1).max()) // P)))
    MU = max(1, int(-(-int(modality_mask.sum(axis=1).max()) // P)))
    LG, MG = LU * P, MU * P

    NC1, NC2 = H + 1, NO
    in_maps = []
    for c in range(NCORES):
        m = {
            "txtT": np.empty((BPC, H, L), BF16),
            "crit": np.empty((BPC, H, LG + MG), BF16),
            "gath": np.empty((BPC, P, LU * NC1 + MU * NC2), BF16),
            "eb1": np.empty((BPC, P, MU), np.float32),
        }
        for b in range(BPC):
            g = BPC * c + b
            tm, mmk = text_mask[g], modality_mask[g]
            pl = np.argsort(1 - tm, kind="stable")[:LG]
            pm = np.argsort(1 - mmk, kind="stable")[:MG]
            tg = text[g][pl]                      # (LG, H)
            mg_ = modality[g][pm]                 # (MG, H)
            eb2 = np.exp(tg @ wt + (tm[pl] - 1.0) * NEGB)       # (LG,)
            eb1 = np.exp(mg_ @ wm + (mmk[pm] - 1.0) * NEGB)     # (MG,)
            m["eb1"][b] = eb1.reshape(MU, P).T
            m["txtT"][b] = (text[g] * wtm).T.astype(BF16)
            m["crit"][b, :, :MG] = mg_.T.astype(BF16)
            m["crit"][b, :, MG:] = (tg * wtm).T.astype(BF16)
            tg1 = np.concatenate([tg, np.ones((LG, 1), np.float32)],
                                 axis=1) * eb2[:, None]
            m["gath"][b, :, : LU * NC1] = tg1.reshape(
                LU, P, NC1).transpose(1, 0, 2).reshape(P, LU * NC1)
            mw = np.zeros((MG, NC2), np.float32)
            mw[:, :H] = mg_ * eb1[:, None]
            mw[:, 2 * H] = eb1
            m["gath"][b, :, LU * NC1 :] = mw.reshape(
                MU, P, NC2).transpose(1, 0, 2).reshape(P, MU * NC2)
        in_maps.append(m)
    return in_maps, MU, LU


def kernel(text, modality, text_mask, modality_mask,
           text_weight, modality_weight, text_modality_weight, bias,
           trace=False):
    from concourse.bass_utils import run_bass_kernel_spmd

    text = np.ascontiguousarray(np.asarray(text, dtype=np.float32))
    in_maps, MU, LU = make_in_maps(text, modality, text_mask, modality_mask,
                                   text_weight, modality_weight,
                                   text_modality_weight)
    nc = get_nc(MU, LU)
    res = run_bass_kernel_spmd(nc, in_maps, core_ids=list(range(NCORES)),
                               trace=trace)
    # Unshard: device rows are (p, j) -> l = j*128 + p; divide by D1 and
    # assemble [text, a, t*a, t*b] on the host.
    outs = []
    for cidx, r in enumerate(res.results):
        raw = np.transpose(r["out_ab"].astype(np.float32),
                           (0, 2, 1, 3)).reshape(BPC, L, NO)
        sl = slice(BPC * cidx, BPC * (cidx + 1))
        ab = raw[:, :, : 2 * H] / raw[:, :, 2 * H : 2 * H + 1]
        t = text[sl]
        outs.append(np.concatenate(
            [t, ab[:, :, :H], t * ab[:, :, :H], t * ab[:, :, H:]], axis=2))
    outp = np.concatenate(outs, axis=0)
    if trace:
        kernel.last_result = res
    return outp


# revision 35
# speedup vs baseline: 1.2683x; 1.2683x over previous
"""BiDAF attention kernel for Trainium2 (8 NeuronCores, data-parallel over batch).

Problem (per full input): B=16, L=M=1024, H=128
  s  = text@tw + (mod@mw).T + (text*tmw)@mod.T + bias          (B, L, M)
  p1 = softmax_M(mmask*s + (1-mmask)*NEG)
  p2 = softmax_L(tmask*s + (1-tmask)*NEG)
  a  = p1 @ mod
  b  = p1 @ p2.T @ text        (computed as p1 @ (p2.T @ text))
  out = [text, a, text*a, text*b]                               (B, L, 4H)

Key facts used:
  * softmax_M is invariant to per-row (per-l) shifts: s0, bias drop from p1;
    softmax_L is invariant to per-column shifts: s1, bias drop from p2.
    The scalar `bias` input therefore cannot affect the output at all.
  * exp(s2 + b) factors: the per-l bias b2 = s0 + (tmask-1)*30000 enters p2
    only through q2 = E2.T @ [textg|1], where E2's partition dim (lg) is the
    contraction dim -- so exp(b2) folds into the rhs rows (host pre-scales
    [textg|1] by exp(b2); masked rows become exactly 0).  Likewise the per-m
    bias b1 = s1 + (mmask-1)*30000 enters the final matmul through its lhsT
    partition dim (mg), so exp(b1) folds into the [mod|wq|1] rows (host
    pre-scales; masked m rows become 0; the device-computed wq columns get
    the factor via a fused dual-scalar multiply).  All device exps are then
    bias-free pure exp(s2), which lets chunks share one ACTIVATE (the ~480ns
    fixed cost per ACTIVATE dominates otherwise).
  * a ones-column on the rhs of the q2/final matmuls yields the softmax
    denominators for free; D1 normalization + text*a / text*b are O(L*H)
    epilogue done on the host along with all layout prep (gather/compaction,
    transposes, bf16 casts, w_tm pre-scale).  The device runs the O(L*M(*H))
    work: 4 matmul families and the exps.
  * sparsity: masked m/l rows are compacted away host-side; the device
    computes only ceil(Mu/128) x ceil(Lu/128) chunks.

Device program per batch (matmul operands bf16; exp psum bf16, accum f32):
  E2[lg, mg]  = exp(XgT_c.T @ modTg)           chunk-pair-merged ACTIVATEs
  E1T[mg, l]  = exp(modTg_k.T @ txtT)          chunk-pair-merged ACTIVATEs
  q2[mg, :]   = sum_c E2_c.T @ [textg*eb2|eb2] 25 matmuls N=129
  wq          = q2[:, :H] * (1/D2) * eb1       DVE fused dual-scalar
  [ar|br|D1]  = sum_k E1_k @ [mod*eb1|wq*eb1|eb1]  k-outer over 4-j groups
Raw [ar|br|D1] (bf16) is DMA'd out; host divides by D1 and assembles
[text, a, t*a, t*b].  Each core processes 2 batches; no cross-core comm.
"""

import numpy as np
import ml_dtypes

BF16 = ml_dtypes.bfloat16
B, L, M, H = 16, 1024, 1024, 128
NCORES = 8
BPC = B // NCORES  # batches per core
P = 128
LT = L // P
NEGB = 30000.0
NO = 2 * H + 1  # raw output row: [a_raw | b_raw | D1]

_CACHE = {}


def _build(MU, LU):
    """Per-core Bass program for MU gathered m-chunks / LU gathered l-chunks
    (SPMD: same NEFF on all 8 cores)."""
    from contextlib import ExitStack

    import concourse.bass as bass
    import concourse.mybir as mybir
    import concourse.tile as tile
    from concourse import bacc
    from concourse.bass import ts

    f32 = mybir.dt.float32
    bf16 = mybir.dt.bfloat16
    Exp = mybir.ActivationFunctionType.Exp
    Alu = mybir.AluOpType

    MG, LG = MU * P, LU * P
    NC1 = H + 1      # [textg | 1] * exp(b2)
    NC2 = 2 * H + 1  # [mod | wq | 1] * exp(b1)

    nc = bacc.Bacc(name="bidaf8v3")
    txtT_d = nc.dram_tensor("txtT", (BPC, H, L), bf16, kind="ExternalInput").ap()
    crit_d = nc.dram_tensor("crit", (BPC, H, LG + MG), bf16,
                            kind="ExternalInput").ap()
    gath_d = nc.dram_tensor("gath", (BPC, P, LU * NC1 + MU * NC2), bf16,
                            kind="ExternalInput").ap()
    eb1_d = nc.dram_tensor("eb1", (BPC, P, MU), f32, kind="ExternalInput").ap()
    out_d = nc.dram_tensor("out_ab", (BPC, P, LT, NO), bf16,
                           kind="ExternalOutput").ap()

    with tile.TileContext(nc) as tc, ExitStack() as ctx:
        inp = ctx.enter_context(tc.tile_pool(name="inp", bufs=2))
        ebuf = ctx.enter_context(tc.tile_pool(name="ebuf", bufs=2))
        small = ctx.enter_context(tc.tile_pool(name="small", bufs=2))
        outp = ctx.enter_context(tc.tile_pool(name="outp", bufs=2))
        # One PSUM shape everywhere: (128, 1024) f32 = 2 banks; two tags x
        # 2 bufs = all 8 banks.  Each chunk/qp/pa-group gets its OWN tile so
        # the tile tracker sees precise deps (a shared multi-slot tile
        # serializes consumers against later producers).  Every matmul slice
        # below lands within a single bank.
        ps = ctx.enter_context(tc.tile_pool(name="ps", bufs=2, space="PSUM"))

        # Load the exp spline tables while the input DMAs run.
        dumm = small.tile([P, 1], f32, tag="dummy")
        nc.vector.memset(dumm, 0.0)
        dum2 = small.tile([P, 1], f32, tag="dummy2")
        nc.scalar.activation(dum2, dumm, Exp)

        st = [{} for _ in range(BPC)]
        # Few, large input DMAs so completion-sem lanes stay clean (early
        # consumers must never gate on an unrelated late transfer sharing a
        # lane).  Tiny eb1s go first; then the critical [modTg|xgT] blob,
        # split so E2 chunk 0's operands (modTg + xgT chunk 0) land first.
        # Warm the PE clock gate (HAM) during the input-DMA wait so the
        # first real fills run at 2.4 GHz.
        wsrc = small.tile([P, 64], bf16, tag="wsrc")
        nc.vector.memset(wsrc, 0.0)
        warm = ps.tile([P, 512], f32, tag="qp")
        for _ in range(30):
            nc.tensor.matmul(warm[:64, :64], wsrc, wsrc, start=True, stop=True)

        # Exactly 8 input DMAs, 2 per HWDGE ring + gath/eb1 on SWDGE, so
        # completion-sem lanes stay clean and no consumer gates on an
        # unrelated late transfer.  Critical [modTg|xgT] blobs first.
        NG = LU * NC1 + MU * NC2
        for b in range(BPC):
            d = st[b]
            d["crit"] = inp.tile([P, LG + MG], bf16, tag="crit", name="crit")
            d["modTg"] = d["crit"][:, :MG]
            d["xgT"] = d["crit"][:, MG:]
            (nc.sync if b == 0 else nc.scalar).dma_start(d["crit"], crit_d[b])
        for b in range(BPC):
            d = st[b]
            d["txtT"] = inp.tile([P, L], bf16, tag="txtT", name="txtT")
            (nc.sync if b == 0 else nc.scalar).dma_start(d["txtT"], txtT_d[b])
        for b in range(BPC):
            d = st[b]
            gath = inp.tile([P, NG], bf16, tag="gath", name="gath")
            d["txtg1"] = gath[:, : LU * NC1].rearrange("p (c q) -> p c q",
                                                       q=NC1)
            d["modwq"] = gath[:, LU * NC1 :].rearrange("p (c q) -> p c q",
                                                       q=NC2)
            nc.gpsimd.dma_start(gath, gath_d[b])
            d["eb1"] = small.tile([P, MU], f32, tag="eb1", name="eb1")
            nc.gpsimd.dma_start(d["eb1"], eb1_d[b])

        def e2_chunk(b, c):
            d = st[b]
            sp = ps.tile([P, 1024], f32, tag="g")
            n0 = min(512, MG)
            nc.tensor.matmul(sp[:, :n0], d["xgT"][:, ts(c, P)],
                             d["modTg"][:, :n0], start=True, stop=True)
            if MG > 512:
                nc.tensor.matmul(sp[:, 512:MG], d["xgT"][:, ts(c, P)],
                                 d["modTg"][:, 512:MG], start=True, stop=True)
            nc.scalar.activation(d["E2"][:, c, :], sp[:, :MG], Exp)

        def e1t_chunk(b, k, split=False):
            # split=True: exp the two l-halves as separate tiles/ACTs so
            # final-phase consumers of the first half unblock earlier.
            d = st[b]
            if split:
                for half in range(2):
                    sp = ps.tile([P, 1024], f32, tag="g")
                    nc.tensor.matmul(sp[:, :512], d["modTg"][:, ts(k, P)],
                                     d["txtT"][:, ts(half, 512)],
                                     start=True, stop=True)
                    nc.scalar.activation(
                        d["E1T"][:, k, ts(half, 512)], sp[:, :512], Exp)
                return
            sp = ps.tile([P, 1024], f32, tag="g")
            for half in range(2):
                nc.tensor.matmul(sp[:, ts(half, 512)], d["modTg"][:, ts(k, P)],
                                 d["txtT"][:, ts(half, 512)],
                                 start=True, stop=True)
            nc.scalar.activation(d["E1T"][:, k, :], sp, Exp)

        def exp_phase(b):
            # Interleave E2/E1T chunks: the 2-deep 'g' slot rotation then
            # forces fills in exactly the ACT consumption order, keeping the
            # ACT stream (the spine of this kernel) gap-free.
            d = st[b]
            d["E2"] = ebuf.tile([P, LU, MG], bf16, tag="E2", name="E2")
            d["E1T"] = ebuf.tile([P, MU, L], bf16, tag="E1T", name="E1T")
            for c in range(LU):
                e2_chunk(b, c)
            for k in range(MU):
                e1t_chunk(b, k)

        def q2_k(b, k):
            # q2 = E2.T @ [textg|1]*eb2 ; wq = q2 * (1/D2) * eb1
            d = st[b]
            qp = ps.tile([P, 512], f32, tag="qp")
            for c in range(LU):
                nc.tensor.matmul(qp[:, :NC1], d["E2"][:, c, ts(k, P)],
                                 d["txtg1"][:, c, :],
                                 start=(c == 0), stop=(c == LU - 1))
            rec = small.tile([P, 1], f32, tag="rec2")
            nc.vector.reciprocal(rec, qp[:, H : H + 1])
            nc.vector.tensor_scalar(d["modwq"][:, k, H : 2 * H],
                                    qp[:, :H], rec,
                                    d["eb1"][:, k : k + 1],
                                    op0=Alu.mult, op1=Alu.mult)

        Copy = mybir.ActivationFunctionType.Copy

        def fin_group(b, j, act_copy=False):
            # [ar|br|D1] = E1 @ [mod|wq|1]*eb1 for one j
            d = st[b]
            pa = ps.tile([P, 512], f32, tag="pa")
            for k in range(MU):
                nc.tensor.matmul(pa[:, :NC2], d["E1T"][:, k, ts(j, P)],
                                 d["modwq"][:, k, :],
                                 start=(k == 0), stop=(k == MU - 1))
            if act_copy:
                nc.scalar.activation(d["absb"][:, j, :], pa[:, :NO], Copy)
            else:
                nc.vector.tensor_copy(d["absb"][:, j, :], pa[:, :NO])
            nc.sync.dma_start(out_d[b][:, j, :], d["absb"][:, j, :])

        for b in range(BPC):
            st[b]["absb"] = outp.tile([P, LT, NO], bf16, tag="absb",
                                      name="absb")

        # Chain-pipelined emission: batch 0's q2/final fills PE idle while
        # batch 1's exps run on ACT, and vice versa.
        exp_phase(0)
        for k in range(MU):
            q2_k(0, k)
        exp_phase(1)
        for j in range(LT):
            fin_group(0, j)
        for k in range(MU):
            q2_k(1, k)
        for j in range(LT):
            fin_group(1, j)
    nc.compile()
    return nc


def get_nc(MU, LU):
    key = (MU, LU)
    if key not in _CACHE:
        _CACHE[key] = _build(MU, LU)
    return _CACHE[key]


def make_in_maps(text, modality, text_mask, modality_mask,
                 text_weight, modality_weight, text_modality_weight):
    text = np.ascontiguousarray(np.asarray(text, dtype=np.float32))
    modality = np.ascontiguousarray(np.asarray(modality, dtype=np.float32))
    text_mask = np.asarray(text_mask).astype(np.int32)
    modality_mask = np.asarray(modality_mask).astype(np.int32)
    wt = np.asarray(text_weight, dtype=np.float32).reshape(H)
    wm = np.asarray(modality_weight, dtype=np.float32).reshape(H)
    wtm = np.asarray(text_modality_weight, dtype=np.float32).reshape(H)

    LU = max(1, int(-(-int(text_mask.sum(axis=1).max()) // P)))
    MU = max(1, int(-(-int(modality_mask.sum(axis=1).max()) // P)))
    LG, MG = LU * P, MU * P

    NC1, NC2 = H + 1, NO
    in_maps = []
    for c in range(NCORES):
        m = {
            "txtT": np.empty((BPC, H, L), BF16),
            "crit": np.empty((BPC, H, LG + MG), BF16),
            "gath": np.empty((BPC, P, LU * NC1 + MU * NC2), BF16),
            "eb1": np.empty((BPC, P, MU), np.float32),
        }
        for b in range(BPC):
            g = BPC * c + b
            tm, mmk = text_mask[g], modality_mask[g]
            pl = np.argsort(1 - tm, kind="stable")[:LG]
            pm = np.argsort(1 - mmk, kind="stable")[:MG]
            tg = text[g][pl]                      # (LG, H)
            mg_ = modality[g][pm]                 # (MG, H)
            eb2 = np.exp(tg @ wt + (tm[pl] - 1.0) * NEGB)       # (LG,)
            eb1 = np.exp(mg_ @ wm + (mmk[pm] - 1.0) * NEGB)     # (MG,)
            m["eb1"][b] = eb1.reshape(MU, P).T
            m["txtT"][b] = (text[g] * wtm).T.astype(BF16)
            m["crit"][b, :, :MG] = mg_.T.astype(BF16)
            m["crit"][b, :, MG:] = (tg * wtm).T.astype(BF16)
            tg1 = np.concatenate([tg, np.ones((LG, 1), np.float32)],
                                 axis=1) * eb2[:, None]
            m["gath"][b, :, : LU * NC1] = tg1.reshape(
                LU, P, NC1).transpose(1, 0, 2).reshape(P, LU * NC1)
            mw = np.zeros((MG, NC2), np.float32)
            mw[:, :H] = mg_ * eb1[:, None]
            mw[:, 2 * H] = eb1
            m["gath"][b, :, LU * NC1 :] = mw.reshape(
                MU, P, NC2).transpose(1, 0, 2).reshape(P, MU * NC2)
        in_maps.append(m)
    return in_maps, MU, LU


def kernel(text, modality, text_mask, modality_mask,
           text_weight, modality_weight, text_modality_weight, bias,
           trace=False):
    from concourse.bass_utils import run_bass_kernel_spmd

    text = np.ascontiguousarray(np.asarray(text, dtype=np.float32))
    in_maps, MU, LU = make_in_maps(text, modality, text_mask, modality_mask,
                                   text_weight, modality_weight,
                                   text_modality_weight)
    nc = get_nc(MU, LU)
    res = run_bass_kernel_spmd(nc, in_maps, core_ids=list(range(NCORES)),
                               trace=trace)
    # Unshard: device rows are (p, j) -> l = j*128 + p; divide by D1 and
    # assemble [text, a, t*a, t*b] on the host.
    outs = []
    for cidx, r in enumerate(res.results):
        raw = np.transpose(r["out_ab"].astype(np.float32),
                           (0, 2, 1, 3)).reshape(BPC, L, NO)
        sl = slice(BPC * cidx, BPC * (cidx + 1))
        ab = raw[:, :, : 2 * H] / raw[:, :, 2 * H : 2 * H + 1]
        t = text[sl]
        outs.append(np.concatenate(
            [t, ab[:, :, :H], t * ab[:, :, :H], t * ab[:, :, H:]], axis=2))
    outp = np.concatenate(outs, axis=0)
    if trace:
        kernel.last_result = res
    return outp
